# revision 4
# baseline (speedup 1.0000x reference)
"""AFT-full kernel for Trainium2, SPMD across 8 NeuronCores.

Math (per batch b):
    q = in1 @ Wq.T + bq ; k = in1 @ Wk.T + bk ; v = in2 @ Wv.T + bv
    num = exp(position_biases) @ (exp(k) * v)      # [t, d]
    den = exp(position_biases) @ exp(k)            # [t, d]
    out = sigmoid(q) * num / den

Sharding: pure data parallel — core i computes batch i (BS == 8 == n_cores).
Weights / biases / position_biases are replicated to every core.

Per-core dataflow (all matmuls in bf16, accumulation in f32 PSUM):
  1. Load W{q,k,v}, cast bf16, DMA-xbar-transpose -> wT[i, o] tiles.
  2. Per n-tile: load in1/in2 rows, cast bf16, xbar-transpose -> x1T/x2T
     ([i, n] layout), matmul k/v projections, ACT exp -> ek, DVE mul -> ekv.
  3. Per t-tile: q projection + sigmoid; stream position_biases rows,
     ACT exp -> bf16, xbar-transpose to [s, t] panels; 32 accumulating
     matmuls (num, den) sharing each stationary tile; DVE epilogue
     sigmoid(q) * num * (1/den); DMA out.
"""

import sys

for _p in ("/opt/trn_rl_repo",):
    if _p not in sys.path:
        sys.path.insert(0, _p)

from contextlib import ExitStack

import numpy as np

import concourse.bass as bass
from concourse import bacc
import concourse.tile as tile
from concourse import mybir
from concourse.bass_utils import run_bass_kernel_spmd

P = 128
N = 2048          # sequence length (n == s == t)
D = 512           # d_model
BS = 8            # batch size == number of cores
NT = N // P       # 16 row tiles
KT = D // P       # 4 contraction tiles for projections
F32 = mybir.dt.float32
BF16 = mybir.dt.bfloat16

_NC_CACHE = {}


def build_nc(with_bias: bool) -> bass.Bass:
    nc = bacc.Bacc()

    in1 = nc.declare_dram_parameter("inputs1", [N, D], F32, isOutput=False)
    in2 = nc.declare_dram_parameter("inputs2", [N, D], F32, isOutput=False)
    Wq = nc.declare_dram_parameter("Wq", [D, D], F32, isOutput=False)
    Wk = nc.declare_dram_parameter("Wk", [D, D], F32, isOutput=False)
    Wv = nc.declare_dram_parameter("Wv", [D, D], F32, isOutput=False)
    bq = nc.declare_dram_parameter("bq", [D], F32, isOutput=False)
    bk = nc.declare_dram_parameter("bk", [D], F32, isOutput=False)
    bv = nc.declare_dram_parameter("bv", [D], F32, isOutput=False)
    pb = nc.declare_dram_parameter("position_biases", [N, N], F32, isOutput=False)
    out = nc.declare_dram_parameter("out", [N, D], F32, isOutput=True)

    with ExitStack() as ctx:
        tc = ctx.enter_context(tile.TileContext(nc))

        persist = ctx.enter_context(tc.tile_pool(name="persist", bufs=1))
        # wT[p, w, i_t, o] == W_w[o, i_t*P + p]  (i.e. W^T with i on partitions)
        wT = persist.tile([P, 3, KT, D], BF16)
        # x1T[p, i_t, n] == in1[n, i_t*P + p]
        x1T = persist.tile([P, KT, N], BF16)
        ek_sb = persist.tile([P, NT, D], BF16)     # exp(k), s on partitions
        ekv_sb = persist.tile([P, NT, D], BF16)    # exp(k) * v

        stage = ctx.enter_context(tc.tile_pool(name="stage", bufs=3))

        ones_t = bias_bf = None
        if with_bias:
            const = ctx.enter_context(tc.tile_pool(name="const", bufs=1))
            ones_t = const.tile([1, P], BF16)
            nc.vector.memset(ones_t, 1.0)
            bias_bf = const.tile([1, 3, D], BF16)
            for w_idx, b in enumerate((bq, bk, bv)):
                b_f = stage.tile([1, D], F32, tag="b_f")
                nc.sync.dma_start(out=b_f, in_=b[:])
                nc.vector.tensor_copy(out=bias_bf[:, w_idx, :], in_=b_f)

        # ---- weights: load, cast, transpose ----
        for w_idx, W in enumerate((Wq, Wk, Wv)):
            for o_t in range(KT):
                wnat = stage.tile([P, D], F32, tag="wnat")
                nc.sync.dma_start(out=wnat, in_=W[o_t * P:(o_t + 1) * P, :])
                wbf = stage.tile([P, D], BF16, tag="wbf")
                nc.vector.tensor_copy(out=wbf, in_=wnat)
                for i_t in range(KT):
                    nc.sync.dma_start(
                        out=wT[:, w_idx, i_t, o_t * P:(o_t + 1) * P],
                        in_=wbf[:, i_t * P:(i_t + 1) * P],
                        transpose=True,
                    )

        # ---- k/v projections (need all s-tiles before num/den) ----
        with tc.tile_pool(name="psum_kv", bufs=2, space="PSUM") as psum_kv:
            for n_t in range(NT):
                x1 = stage.tile([P, D], F32, tag="x1")
                nc.sync.dma_start(out=x1, in_=in1[n_t * P:(n_t + 1) * P, :])
                x1b = stage.tile([P, D], BF16, tag="x1b")
                nc.vector.tensor_copy(out=x1b, in_=x1)
                for i_t in range(KT):
                    nc.sync.dma_start(
                        out=x1T[:, i_t, n_t * P:(n_t + 1) * P],
                        in_=x1b[:, i_t * P:(i_t + 1) * P],
                        transpose=True,
                    )

                x2 = stage.tile([P, D], F32, tag="x2")
                nc.sync.dma_start(out=x2, in_=in2[n_t * P:(n_t + 1) * P, :])
                x2b = stage.tile([P, D], BF16, tag="x2b")
                nc.vector.tensor_copy(out=x2b, in_=x2)
                x2T = stage.tile([P, KT, P], BF16, tag="x2T")
                for i_t in range(KT):
                    nc.sync.dma_start(
                        out=x2T[:, i_t, :],
                        in_=x2b[:, i_t * P:(i_t + 1) * P],
                        transpose=True,
                    )

                psk = psum_kv.tile([P, D], F32, tag="psk")
                psv = psum_kv.tile([P, D], F32, tag="psv")
                for i_t in range(KT):
                    nc.tensor.matmul(
                        psk,
                        x1T[:, i_t, n_t * P:(n_t + 1) * P],
                        wT[:, 1, i_t, :],
                        start=(i_t == 0),
                        stop=(i_t == KT - 1 and not with_bias),
                    )
                for i_t in range(KT):
                    nc.tensor.matmul(
                        psv,
                        x2T[:, i_t, :],
                        wT[:, 2, i_t, :],
                        start=(i_t == 0),
                        stop=(i_t == KT - 1 and not with_bias),
                    )
                if with_bias:
                    nc.tensor.matmul(psk, ones_t, bias_bf[:, 1, :],
                                     start=False, stop=True)
                    nc.tensor.matmul(psv, ones_t, bias_bf[:, 2, :],
                                     start=False, stop=True)

                nc.scalar.activation(
                    out=ek_sb[:, n_t, :], in_=psk,
                    func=mybir.ActivationFunctionType.Exp)
                nc.vector.tensor_mul(ekv_sb[:, n_t, :], ek_sb[:, n_t, :], psv)

        psum_q = ctx.enter_context(
            tc.tile_pool(name="psum_q", bufs=2, space="PSUM"))
        psum_nd = ctx.enter_context(
            tc.tile_pool(name="psum_nd", bufs=2, space="PSUM"))

        # ---- per t-tile: q proj, expB panel, num/den, epilogue ----
        for t_t in range(NT):
            psq = psum_q.tile([P, D], F32, tag="psq")
            for i_t in range(KT):
                nc.tensor.matmul(
                    psq,
                    x1T[:, i_t, t_t * P:(t_t + 1) * P],
                    wT[:, 0, i_t, :],
                    start=(i_t == 0),
                    stop=(i_t == KT - 1 and not with_bias),
                )
            if with_bias:
                nc.tensor.matmul(psq, ones_t, bias_bf[:, 0, :],
                                 start=False, stop=True)
            qsig = stage.tile([P, D], BF16, tag="qsig")
            nc.scalar.activation(
                out=qsig, in_=psq, func=mybir.ActivationFunctionType.Sigmoid)

            pbrow = stage.tile([P, N], F32, tag="pbrow")
            nc.sync.dma_start(out=pbrow, in_=pb[t_t * P:(t_t + 1) * P, :])
            pbexp = stage.tile([P, N], BF16, tag="pbexp")
            nc.scalar.activation(
                out=pbexp, in_=pbrow, func=mybir.ActivationFunctionType.Exp)
            # panel[p, s_t, t] == expB[t_t*P + t, s_t*P + p]
            panel = stage.tile([P, NT, P], BF16, tag="panel")
            for s_t in range(NT):
                nc.sync.dma_start(
                    out=panel[:, s_t, :],
                    in_=pbexp[:, s_t * P:(s_t + 1) * P],
                    transpose=True,
                )

            pnum = psum_nd.tile([P, D], F32, tag="pnum")
            pden = psum_nd.tile([P, D], F32, tag="pden")
            for s_t in range(NT):
                lhsT = panel[:, s_t, :]
                nc.tensor.matmul(pnum, lhsT, ekv_sb[:, s_t, :],
                                 start=(s_t == 0), stop=(s_t == NT - 1))
                nc.tensor.matmul(pden, lhsT, ek_sb[:, s_t, :],
                                 start=(s_t == 0), stop=(s_t == NT - 1))

            rec = stage.tile([P, D], F32, tag="rec")
            nc.vector.reciprocal(out=rec, in_=pden)
            rat = stage.tile([P, D], F32, tag="rat")
            nc.vector.tensor_mul(rat, rec, pnum)
            outt = stage.tile([P, D], F32, tag="outt")
            nc.vector.tensor_mul(outt, rat, qsig)
            nc.sync.dma_start(
                out=out[t_t * P:(t_t + 1) * P, :], in_=outt)

    nc.finalize()
    return nc


def _get_nc(with_bias: bool) -> bass.Bass:
    key = with_bias
    if key not in _NC_CACHE:
        _NC_CACHE[key] = build_nc(with_bias)
    return _NC_CACHE[key]


def _make_in_maps(inputs: dict) -> list[dict]:
    in1 = np.ascontiguousarray(inputs["inputs1"], dtype=np.float32)
    in2 = np.ascontiguousarray(inputs["inputs2"], dtype=np.float32)
    shared = {
        k: np.ascontiguousarray(inputs[k], dtype=np.float32)
        for k in ("Wq", "Wk", "Wv", "bq", "bk", "bv", "position_biases")
    }
    return [
        {"inputs1": in1[c], "inputs2": in2[c], **shared}
        for c in range(BS)
    ]


def run(inputs: dict, trace: bool = False):
    """Returns (out [8,2048,512] f32, exec_time_ns or None)."""
    with_bias = any(
        np.any(np.asarray(inputs[b])) for b in ("bq", "bk", "bv"))
    nc = _get_nc(with_bias)
    in_maps = _make_in_maps(inputs)
    res = run_bass_kernel_spmd(
        nc, in_maps, core_ids=list(range(BS)), trace=trace)
    out = np.stack(
        [np.asarray(res.results[c]["out"]) for c in range(BS)], axis=0)
    return out.astype(np.float32), res.exec_time_ns


def kernel(**inputs) -> np.ndarray:
    out, _ = run(inputs, trace=False)
    return out


# revision 5
# speedup vs baseline: 2.0575x; 2.0575x over previous
"""AFT-full kernel for Trainium2, SPMD across 8 NeuronCores.

Math (per batch b):
    q = in1 @ Wq.T + bq ; k = in1 @ Wk.T + bk ; v = in2 @ Wv.T + bv
    num = exp(position_biases) @ (exp(k) * v)      # [t, d]
    den = exp(position_biases) @ exp(k)            # [t, d]
    out = sigmoid(q) * num / den

Sharding: pure data parallel — core i computes batch i (BS == 8 == n_cores).
Weights / biases / position_biases are replicated to every core.

Per-core dataflow (matmuls in bf16, accumulation in f32 PSUM):
  1. Load W{q,k,v}, cast bf16, one batched DMA-xbar transpose per [128, 512]
     tile -> wT[i, o] layout.
  2. Per n-tile: load in1/in2 rows, cast bf16, batched xbar transpose ->
     x1T/x2T ([i, n] layout), matmul k/v projections, ACT exp -> ek,
     DVE mul -> ekv.
  3. Per t-tile: q projection + sigmoid; stream position_biases rows,
     ACT exp -> bf16, ONE batched xbar transpose -> [s, t] panel; 32
     accumulating matmuls (num, den) sharing each stationary tile; DVE
     epilogue sigmoid(q) * num * approx(1/den); DMA out.

DMA issue split: SP (nc.sync) takes plain loads/stores + the 16 panel
transposes; ACT (nc.scalar) takes the x/w transposes, so neither HWDGE
issuer serializes the kernel (lesson from the first profile: 432 separate
transpose instructions on SP cost 530us of issue time).
"""

import sys

for _p in ("/opt/trn_rl_repo",):
    if _p not in sys.path:
        sys.path.insert(0, _p)

from contextlib import ExitStack

import numpy as np

import concourse.bass as bass
from concourse import bacc
import concourse.tile as tile
from concourse import mybir
from concourse.bass_utils import run_bass_kernel_spmd

P = 128
N = 2048          # sequence length (n == s == t)
D = 512           # d_model
BS = 8            # batch size == number of cores
NT = N // P       # 16 row tiles
KT = D // P       # 4 contraction tiles for projections
F32 = mybir.dt.float32
BF16 = mybir.dt.bfloat16

_NC_CACHE = {}


def build_nc(with_bias: bool) -> bass.Bass:
    nc = bacc.Bacc()

    in1 = nc.declare_dram_parameter("inputs1", [N, D], F32, isOutput=False)
    in2 = nc.declare_dram_parameter("inputs2", [N, D], F32, isOutput=False)
    Wq = nc.declare_dram_parameter("Wq", [D, D], F32, isOutput=False)
    Wk = nc.declare_dram_parameter("Wk", [D, D], F32, isOutput=False)
    Wv = nc.declare_dram_parameter("Wv", [D, D], F32, isOutput=False)
    bq = nc.declare_dram_parameter("bq", [D], F32, isOutput=False)
    bk = nc.declare_dram_parameter("bk", [D], F32, isOutput=False)
    bv = nc.declare_dram_parameter("bv", [D], F32, isOutput=False)
    pb = nc.declare_dram_parameter("position_biases", [N, N], F32, isOutput=False)
    out = nc.declare_dram_parameter("out", [N, D], F32, isOutput=True)

    with ExitStack() as ctx:
        tc = ctx.enter_context(tile.TileContext(nc))

        persist = ctx.enter_context(tc.tile_pool(name="persist", bufs=1))
        # wT[p, w, i_t, o] == W_w[o, i_t*P + p]  (i.e. W^T with i on partitions)
        wT = persist.tile([P, 3, KT, D], BF16)
        # x1T[p, i_t, n] == in1[n, i_t*P + p]
        x1T = persist.tile([P, KT, N], BF16)
        ek_sb = persist.tile([P, NT, D], BF16)     # exp(k), s on partitions
        ekv_sb = persist.tile([P, NT, D], BF16)    # exp(k) * v

        stage = ctx.enter_context(tc.tile_pool(name="stage", bufs=3))

        ones_t = bias_bf = None
        if with_bias:
            const = ctx.enter_context(tc.tile_pool(name="const", bufs=1))
            ones_t = const.tile([1, P], BF16)
            nc.vector.memset(ones_t, 1.0)
            bias_bf = const.tile([1, 3, D], BF16)
            for w_idx, b in enumerate((bq, bk, bv)):
                b_f = stage.tile([1, D], F32, tag="b_f")
                nc.sync.dma_start(out=b_f, in_=b[:])
                nc.vector.tensor_copy(out=bias_bf[:, w_idx, :], in_=b_f)

        # ---- weights: load, cast, batched transpose ----
        for w_idx, W in enumerate((Wq, Wk, Wv)):
            for o_t in range(KT):
                wnat = stage.tile([P, D], F32, tag="wnat")
                nc.sync.dma_start(out=wnat, in_=W[o_t * P:(o_t + 1) * P, :])
                wbf = stage.tile([P, D], BF16, tag="wbf")
                nc.vector.tensor_copy(out=wbf, in_=wnat)
                # wT[p, w_idx, i_t, o_t*P+f] = wbf[f, i_t*P+p]
                nc.scalar.dma_start(
                    out=wT[:, w_idx, :, o_t * P:(o_t + 1) * P],
                    in_=wbf[:, :],
                    transpose=True,
                )

        # ---- k/v projections (need all s-tiles before num/den) ----
        with tc.tile_pool(name="psum_kv", bufs=2, space="PSUM") as psum_kv:
            for n_t in range(NT):
                x1 = stage.tile([P, D], F32, tag="x1")
                nc.sync.dma_start(out=x1, in_=in1[n_t * P:(n_t + 1) * P, :])
                x1b = stage.tile([P, D], BF16, tag="x1b")
                nc.vector.tensor_copy(out=x1b, in_=x1)
                nc.scalar.dma_start(
                    out=x1T[:, :, n_t * P:(n_t + 1) * P],
                    in_=x1b[:, :],
                    transpose=True,
                )

                x2 = stage.tile([P, D], F32, tag="x2")
                nc.sync.dma_start(out=x2, in_=in2[n_t * P:(n_t + 1) * P, :])
                x2b = stage.tile([P, D], BF16, tag="x2b")
                nc.vector.tensor_copy(out=x2b, in_=x2)
                x2T = stage.tile([P, KT, P], BF16, tag="x2T")
                nc.scalar.dma_start(
                    out=x2T[:, :, :],
                    in_=x2b[:, :],
                    transpose=True,
                )

                psk = psum_kv.tile([P, D], F32, tag="psk")
                psv = psum_kv.tile([P, D], F32, tag="psv")
                for i_t in range(KT):
                    nc.tensor.matmul(
                        psk,
                        x1T[:, i_t, n_t * P:(n_t + 1) * P],
                        wT[:, 1, i_t, :],
                        start=(i_t == 0),
                        stop=(i_t == KT - 1 and not with_bias),
                    )
                for i_t in range(KT):
                    nc.tensor.matmul(
                        psv,
                        x2T[:, i_t, :],
                        wT[:, 2, i_t, :],
                        start=(i_t == 0),
                        stop=(i_t == KT - 1 and not with_bias),
                    )
                if with_bias:
                    nc.tensor.matmul(psk, ones_t, bias_bf[:, 1, :],
                                     start=False, stop=True)
                    nc.tensor.matmul(psv, ones_t, bias_bf[:, 2, :],
                                     start=False, stop=True)

                nc.scalar.activation(
                    out=ek_sb[:, n_t, :], in_=psk,
                    func=mybir.ActivationFunctionType.Exp)
                nc.vector.tensor_mul(ekv_sb[:, n_t, :], ek_sb[:, n_t, :], psv)

        psum_q = ctx.enter_context(
            tc.tile_pool(name="psum_q", bufs=2, space="PSUM"))
        psum_nd = ctx.enter_context(
            tc.tile_pool(name="psum_nd", bufs=2, space="PSUM"))

        # ---- per t-tile: q proj, expB panel, num/den, epilogue ----
        for t_t in range(NT):
            psq = psum_q.tile([P, D], F32, tag="psq")
            for i_t in range(KT):
                nc.tensor.matmul(
                    psq,
                    x1T[:, i_t, t_t * P:(t_t + 1) * P],
                    wT[:, 0, i_t, :],
                    start=(i_t == 0),
                    stop=(i_t == KT - 1 and not with_bias),
                )
            if with_bias:
                nc.tensor.matmul(psq, ones_t, bias_bf[:, 0, :],
                                 start=False, stop=True)
            qsig = stage.tile([P, D], BF16, tag="qsig")
            nc.scalar.activation(
                out=qsig, in_=psq, func=mybir.ActivationFunctionType.Sigmoid)

            pbrow = stage.tile([P, N], F32, tag="pbrow")
            nc.sync.dma_start(out=pbrow, in_=pb[t_t * P:(t_t + 1) * P, :])
            pbexp = stage.tile([P, N], BF16, tag="pbexp")
            nc.scalar.activation(
                out=pbexp, in_=pbrow, func=mybir.ActivationFunctionType.Exp)
            # panel[p, s_t, t] == expB[t_t*P + t, s_t*P + p]
            panel = stage.tile([P, NT, P], BF16, tag="panel")
            nc.sync.dma_start(out=panel[:, :, :], in_=pbexp[:, :],
                              transpose=True)

            pnum = psum_nd.tile([P, D], F32, tag="pnum")
            pden = psum_nd.tile([P, D], F32, tag="pden")
            for s_t in range(NT):
                lhsT = panel[:, s_t, :]
                nc.tensor.matmul(pnum, lhsT, ekv_sb[:, s_t, :],
                                 start=(s_t == 0), stop=(s_t == NT - 1))
                nc.tensor.matmul(pden, lhsT, ek_sb[:, s_t, :],
                                 start=(s_t == 0), stop=(s_t == NT - 1))

            rec = stage.tile([P, D], F32, tag="rec")
            nc.vector.reciprocal_approx_fast(out=rec, in_=pden)
            rat = stage.tile([P, D], F32, tag="rat")
            nc.vector.tensor_mul(rat, rec, pnum)
            outt = stage.tile([P, D], F32, tag="outt")
            nc.vector.tensor_mul(outt, rat, qsig)
            nc.sync.dma_start(
                out=out[t_t * P:(t_t + 1) * P, :], in_=outt)

    nc.finalize()
    return nc


def _get_nc(with_bias: bool) -> bass.Bass:
    key = with_bias
    if key not in _NC_CACHE:
        _NC_CACHE[key] = build_nc(with_bias)
    return _NC_CACHE[key]


def _make_in_maps(inputs: dict) -> list[dict]:
    in1 = np.ascontiguousarray(inputs["inputs1"], dtype=np.float32)
    in2 = np.ascontiguousarray(inputs["inputs2"], dtype=np.float32)
    shared = {
        k: np.ascontiguousarray(inputs[k], dtype=np.float32)
        for k in ("Wq", "Wk", "Wv", "bq", "bk", "bv", "position_biases")
    }
    return [
        {"inputs1": in1[c], "inputs2": in2[c], **shared}
        for c in range(BS)
    ]


def run(inputs: dict, trace: bool = False):
    """Returns (out [8,2048,512] f32, exec_time_ns or None)."""
    with_bias = any(
        np.any(np.asarray(inputs[b])) for b in ("bq", "bk", "bv"))
    nc = _get_nc(with_bias)
    in_maps = _make_in_maps(inputs)
    res = run_bass_kernel_spmd(
        nc, in_maps, core_ids=list(range(BS)), trace=trace)
    out = np.stack(
        [np.asarray(res.results[c]["out"]) for c in range(BS)], axis=0)
    return out.astype(np.float32), res.exec_time_ns


def kernel(**inputs) -> np.ndarray:
    out, _ = run(inputs, trace=False)
    return out


# revision 9
# speedup vs baseline: 2.6610x; 1.2933x over previous
"""AFT-full kernel for Trainium2, SPMD across 8 NeuronCores.

Math (per batch b):
    q = in1 @ Wq.T + bq ; k = in1 @ Wk.T + bk ; v = in2 @ Wv.T + bv
    num = exp(position_biases) @ (exp(k) * v)      # [t, d]
    den = exp(position_biases) @ exp(k)            # [t, d]
    out = sigmoid(q) * num / den

Sharding: pure data parallel — core i computes batch i (BS == 8 == n_cores).
Weights / biases / position_biases are replicated to every core.

Per-core dataflow (matmuls in bf16, accumulation in f32 PSUM):
  - GpSimd SWDGE cast-loads inputs f32->bf16 in groups of 4 row tiles.
  - Batched DMA-xbar transposes (SP-issued, contiguous destinations,
    out[p, m, f] = in[f, m*128+p]) produce [contraction on partitions]
    layouts for x, W and the exp(position_biases) panels.
  - Phase order KV -> Q -> T keeps the ScalarE LUT swaps to 3 (exp,
    sigmoid, exp) and lets Q's matmuls absorb the KV->T latency bubble.
  - The position_biases pipeline (load f32 rows, ACT exp -> bf16, panel
    transpose) runs PRE=2 tiles ahead of the num/den consumer loop.
  - Tiles read by DMA-transpose get enough pool slots that slots are
    never recycled while a transpose may still be reading (HW xbar
    completion vs slot-reuse WAR race seen on silicon).
"""

import sys

for _p in ("/opt/trn_rl_repo",):
    if _p not in sys.path:
        sys.path.insert(0, _p)

from contextlib import ExitStack

import numpy as np

import concourse.bass as bass
from concourse import bacc
import concourse.tile as tile
from concourse import mybir
from concourse.bass_utils import run_bass_kernel_spmd

P = 128
N = 2048          # sequence length (n == s == t)
D = 512           # d_model
BS = 8            # batch size == number of cores
NT = N // P       # 16 row tiles
KT = D // P       # 4 contraction tiles for projections
XG = 4            # x row-tiles loaded per group
NG = NT // XG     # 4 groups
PRE = 2           # position-bias pipeline depth (tiles ahead)
F32 = mybir.dt.float32
BF16 = mybir.dt.bfloat16

_NC_CACHE = {}


def build_nc(with_bias: bool) -> bass.Bass:
    nc = bacc.Bacc()

    in1 = nc.declare_dram_parameter("inputs1", [N, D], F32, isOutput=False)
    in2 = nc.declare_dram_parameter("inputs2", [N, D], F32, isOutput=False)
    Wq = nc.declare_dram_parameter("Wq", [D, D], F32, isOutput=False)
    Wk = nc.declare_dram_parameter("Wk", [D, D], F32, isOutput=False)
    Wv = nc.declare_dram_parameter("Wv", [D, D], F32, isOutput=False)
    bq = nc.declare_dram_parameter("bq", [D], F32, isOutput=False)
    bk = nc.declare_dram_parameter("bk", [D], F32, isOutput=False)
    bv = nc.declare_dram_parameter("bv", [D], F32, isOutput=False)
    pb = nc.declare_dram_parameter("position_biases", [N, N], F32, isOutput=False)
    out = nc.declare_dram_parameter("out", [N, D], F32, isOutput=True)

    with ExitStack() as ctx:
        tc = ctx.enter_context(tile.TileContext(nc))

        persist = ctx.enter_context(tc.tile_pool(name="persist", bufs=1))
        # wT[p, w, o_t, i_t, f] == W_w[o_t*P + f, i_t*P + p]
        # (W^T with i on partitions; o split (o_t, f) so each weight
        # transpose writes a contiguous [P, KT, P] block)
        wT = persist.tile([P, 3, KT, KT, P], BF16)
        # x1T[p, g, m, f] == in1[g*(XG*P) + (m//KT)*P + f, (m%KT)*P + p]
        x1T = persist.tile([P, NG, XG * KT, P], BF16)
        ek_sb = persist.tile([P, NT, D], BF16)     # exp(k), s on partitions
        ekv_sb = persist.tile([P, NT, D], BF16)    # exp(k) * v
        qsig_sb = persist.tile([P, NT, D], BF16)   # sigmoid(q), t on partitions

        stage = ctx.enter_context(tc.tile_pool(name="stage", bufs=2))
        # tiles read by DMA transposes: one slot per use, never recycled hot
        wpool = ctx.enter_context(tc.tile_pool(name="wpool", bufs=12))
        xpool = ctx.enter_context(tc.tile_pool(name="xpool", bufs=3))
        pbpool = ctx.enter_context(tc.tile_pool(name="pbpool", bufs=PRE + 1))

        ones_t = bias_bf = None
        if with_bias:
            const = ctx.enter_context(tc.tile_pool(name="const", bufs=1))
            ones_t = const.tile([1, P], BF16)
            nc.vector.memset(ones_t, 1.0)
            bias_bf = const.tile([1, 3, D], BF16)
            for w_idx, b in enumerate((bq, bk, bv)):
                nc.gpsimd.dma_start(out=bias_bf[:, w_idx, :], in_=b[:])

        # ---- weights: load, cast, batched transpose ----
        for w_idx, W in enumerate((Wq, Wk, Wv)):
            for o_t in range(KT):
                wnat = stage.tile([P, D], F32, tag="wnat")
                nc.sync.dma_start(out=wnat, in_=W[o_t * P:(o_t + 1) * P, :])
                wbf = wpool.tile([P, D], BF16, tag="wbf")
                nc.vector.tensor_copy(out=wbf, in_=wnat)
                nc.sync.dma_start(
                    out=wT[:, w_idx, o_t, :, :],
                    in_=wbf[:, :],
                    transpose=True,
                )

        def x1t_lhs(n_t, i_t):
            g, a = divmod(n_t, XG)
            return x1T[:, g, a * KT + i_t, :]

        # ---- position-bias panel pipeline ----
        panels = {}

        def pb_stage(t_t):
            pbrow = pbpool.tile([P, N], F32, tag="pbrow")
            nc.sync.dma_start(out=pbrow, in_=pb[t_t * P:(t_t + 1) * P, :])
            pbexp = pbpool.tile([P, N], BF16, tag="pbexp")
            nc.scalar.activation(
                out=pbexp, in_=pbrow, func=mybir.ActivationFunctionType.Exp)
            # panel[p, s_t, f] == expB[t_t*P + f, s_t*P + p]
            panel = pbpool.tile([P, NT, P], BF16, tag="panel")
            nc.sync.dma_start(out=panel, in_=pbexp[:, :], transpose=True)
            panels[t_t] = panel

        for t_t in range(PRE):
            pb_stage(t_t)

        # ---- phase KV: x loads, k/v projections, exp, ekv ----
        with tc.tile_pool(name="psum_kv", bufs=2, space="PSUM") as psum_kv:
            for g in range(NG):
                x1b = xpool.tile([P, XG, D], BF16, tag="x1b")
                nc.gpsimd.dma_start(
                    out=x1b,
                    in_=in1[g * XG * P:(g + 1) * XG * P, :].rearrange(
                        "(a p) d -> p a d", p=P),
                )
                nc.sync.dma_start(
                    out=x1T[:, g, :, :], in_=x1b[:, :, :], transpose=True)

                x2b = xpool.tile([P, XG, D], BF16, tag="x2b")
                nc.gpsimd.dma_start(
                    out=x2b,
                    in_=in2[g * XG * P:(g + 1) * XG * P, :].rearrange(
                        "(a p) d -> p a d", p=P),
                )
                x2T = xpool.tile([P, XG * KT, P], BF16, tag="x2T")
                nc.sync.dma_start(
                    out=x2T, in_=x2b[:, :, :], transpose=True)

                for a in range(XG):
                    n_t = g * XG + a
                    psk = psum_kv.tile([P, D], F32, tag="psk")
                    psv = psum_kv.tile([P, D], F32, tag="psv")
                    for i_t in range(KT):
                        nc.tensor.matmul(
                            psk,
                            x1t_lhs(n_t, i_t),
                            wT[:, 1, :, i_t, :],
                            start=(i_t == 0),
                            stop=(i_t == KT - 1 and not with_bias),
                        )
                    for i_t in range(KT):
                        nc.tensor.matmul(
                            psv,
                            x2T[:, a * KT + i_t, :],
                            wT[:, 2, :, i_t, :],
                            start=(i_t == 0),
                            stop=(i_t == KT - 1 and not with_bias),
                        )
                    if with_bias:
                        nc.tensor.matmul(psk, ones_t, bias_bf[:, 1, :],
                                         start=False, stop=True)
                        nc.tensor.matmul(psv, ones_t, bias_bf[:, 2, :],
                                         start=False, stop=True)

                    nc.scalar.activation(
                        out=ek_sb[:, n_t, :], in_=psk,
                        func=mybir.ActivationFunctionType.Exp)
                    nc.vector.tensor_mul(
                        ekv_sb[:, n_t, :], ek_sb[:, n_t, :], psv)

        # ---- phase Q: q projections + sigmoid ----
        with tc.tile_pool(name="psum_q", bufs=2, space="PSUM") as psum_q:
            for n_t in range(NT):
                psq = psum_q.tile([P, D], F32, tag="psq")
                for i_t in range(KT):
                    nc.tensor.matmul(
                        psq,
                        x1t_lhs(n_t, i_t),
                        wT[:, 0, :, i_t, :],
                        start=(i_t == 0),
                        stop=(i_t == KT - 1 and not with_bias),
                    )
                if with_bias:
                    nc.tensor.matmul(psq, ones_t, bias_bf[:, 0, :],
                                     start=False, stop=True)
                nc.scalar.activation(
                    out=qsig_sb[:, n_t, :], in_=psq,
                    func=mybir.ActivationFunctionType.Sigmoid)

        psum_nd = ctx.enter_context(
            tc.tile_pool(name="psum_nd", bufs=3, space="PSUM"))

        # ---- phase T: num/den + epilogue per t-tile ----
        for t_t in range(NT):
            if t_t + PRE < NT:
                pb_stage(t_t + PRE)
            panel = panels.pop(t_t)

            pnum = psum_nd.tile([P, D], F32, tag="pnum")
            pden = psum_nd.tile([P, D], F32, tag="pden")
            for s_t in range(NT):
                lhsT = panel[:, s_t, :]
                nc.tensor.matmul(pnum, lhsT, ekv_sb[:, s_t, :],
                                 start=(s_t == 0), stop=(s_t == NT - 1))
                nc.tensor.matmul(pden, lhsT, ek_sb[:, s_t, :],
                                 start=(s_t == 0), stop=(s_t == NT - 1))

            rec = stage.tile([P, D], F32, tag="rec")
            nc.vector.reciprocal_approx_fast(out=rec, in_=pden)
            rat = stage.tile([P, D], F32, tag="rat")
            nc.vector.tensor_mul(rat, rec, pnum)
            outt = stage.tile([P, D], F32, tag="outt")
            nc.vector.tensor_mul(outt, rat, qsig_sb[:, t_t, :])
            nc.sync.dma_start(
                out=out[t_t * P:(t_t + 1) * P, :], in_=outt)

    nc.finalize()
    return nc


def _get_nc(with_bias: bool) -> bass.Bass:
    key = with_bias
    if key not in _NC_CACHE:
        _NC_CACHE[key] = build_nc(with_bias)
    return _NC_CACHE[key]


def _make_in_maps(inputs: dict) -> list[dict]:
    in1 = np.ascontiguousarray(inputs["inputs1"], dtype=np.float32)
    in2 = np.ascontiguousarray(inputs["inputs2"], dtype=np.float32)
    shared = {
        k: np.ascontiguousarray(inputs[k], dtype=np.float32)
        for k in ("Wq", "Wk", "Wv", "bq", "bk", "bv", "position_biases")
    }
    return [
        {"inputs1": in1[c], "inputs2": in2[c], **shared}
        for c in range(BS)
    ]


def run(inputs: dict, trace: bool = False):
    """Returns (out [8,2048,512] f32, exec_time_ns or None)."""
    with_bias = any(
        np.any(np.asarray(inputs[b])) for b in ("bq", "bk", "bv"))
    nc = _get_nc(with_bias)
    in_maps = _make_in_maps(inputs)
    res = run_bass_kernel_spmd(
        nc, in_maps, core_ids=list(range(BS)), trace=trace)
    out = np.stack(
        [np.asarray(res.results[c]["out"]) for c in range(BS)], axis=0)
    return out.astype(np.float32), res.exec_time_ns


def kernel(**inputs) -> np.ndarray:
    out, _ = run(inputs, trace=False)
    return out


# revision 12
# speedup vs baseline: 2.6722x; 1.0042x over previous
"""AFT-full kernel for Trainium2, SPMD across 8 NeuronCores.

Math (per batch b):
    q = in1 @ Wq.T + bq ; k = in1 @ Wk.T + bk ; v = in2 @ Wv.T + bv
    num = exp(position_biases) @ (exp(k) * v)      # [t, d]
    den = exp(position_biases) @ exp(k)            # [t, d]
    out = sigmoid(q) * num / den

Sharding: pure data parallel — core i computes batch i (BS == 8 == n_cores).
Weights / biases / position_biases are replicated to every core.

Per-core dataflow (matmuls in bf16, accumulation in f32 PSUM):
  - GpSimd SWDGE cast-loads inputs f32->bf16 in groups of 4 row tiles.
  - Batched DMA-xbar transposes (SP-issued, contiguous destinations,
    out[p, m, f] = in[f, m*128+p]) produce [contraction on partitions]
    layouts for x, W and the exp(position_biases) panels.
  - Phase order KV -> Q -> T keeps the ScalarE LUT swaps to 3 (exp,
    sigmoid, exp) and lets Q's matmuls absorb the KV->T latency bubble.
  - The position_biases pipeline (load f32 rows, ACT exp -> bf16, panel
    transpose) runs PRE=2 tiles ahead of the num/den consumer loop.
  - Tiles read by DMA-transpose get enough pool slots that slots are
    never recycled while a transpose may still be reading (HW xbar
    completion vs slot-reuse WAR race seen on silicon).
"""

import sys

for _p in ("/opt/trn_rl_repo",):
    if _p not in sys.path:
        sys.path.insert(0, _p)

from contextlib import ExitStack

import numpy as np

import concourse.bass as bass
from concourse import bacc
import concourse.tile as tile
from concourse import mybir
from concourse.bass_utils import run_bass_kernel_spmd

P = 128
N = 2048          # sequence length (n == s == t)
D = 512           # d_model
BS = 8            # batch size == number of cores
NT = N // P       # 16 row tiles
KT = D // P       # 4 contraction tiles for projections
XG = 4            # x row-tiles loaded per group
NG = NT // XG     # 4 groups
PRE = 2           # position-bias pipeline depth (tiles ahead)
F32 = mybir.dt.float32
BF16 = mybir.dt.bfloat16

_NC_CACHE = {}


def build_nc(with_bias: bool) -> bass.Bass:
    nc = bacc.Bacc()

    in1 = nc.declare_dram_parameter("inputs1", [N, D], F32, isOutput=False)
    in2 = nc.declare_dram_parameter("inputs2", [N, D], F32, isOutput=False)
    Wq = nc.declare_dram_parameter("Wq", [D, D], F32, isOutput=False)
    Wk = nc.declare_dram_parameter("Wk", [D, D], F32, isOutput=False)
    Wv = nc.declare_dram_parameter("Wv", [D, D], F32, isOutput=False)
    bq = nc.declare_dram_parameter("bq", [D], F32, isOutput=False)
    bk = nc.declare_dram_parameter("bk", [D], F32, isOutput=False)
    bv = nc.declare_dram_parameter("bv", [D], F32, isOutput=False)
    pb = nc.declare_dram_parameter("position_biases", [N, N], F32, isOutput=False)
    out = nc.declare_dram_parameter("out", [N, D], F32, isOutput=True)

    with ExitStack() as ctx:
        tc = ctx.enter_context(tile.TileContext(nc))

        persist = ctx.enter_context(tc.tile_pool(name="persist", bufs=1))
        # wT[p, w, o_t, i_t, f] == W_w[o_t*P + f, i_t*P + p]
        # (W^T with i on partitions; o split (o_t, f) so each weight
        # transpose writes a contiguous [P, KT, P] block)
        wT = persist.tile([P, 3, KT, KT, P], BF16)
        # x1T[p, g, m, f] == in1[g*(XG*P) + (m//KT)*P + f, (m%KT)*P + p]
        x1T = persist.tile([P, NG, XG * KT, P], BF16)
        ek_sb = persist.tile([P, NT, D], BF16)     # exp(k), s on partitions
        ekv_sb = persist.tile([P, NT, D], BF16)    # exp(k) * v
        qsig_sb = persist.tile([P, NT, D], BF16)   # sigmoid(q), t on partitions

        stage = ctx.enter_context(tc.tile_pool(name="stage", bufs=2))
        # tiles read by DMA transposes: one slot per use, never recycled hot
        wpool = ctx.enter_context(tc.tile_pool(name="wpool", bufs=12))
        xpool = ctx.enter_context(tc.tile_pool(name="xpool", bufs=3))
        pbpool = ctx.enter_context(tc.tile_pool(name="pbpool", bufs=PRE + 1))

        ones_t = bias_bf = None
        if with_bias:
            const = ctx.enter_context(tc.tile_pool(name="const", bufs=1))
            ones_t = const.tile([1, P], BF16)
            nc.vector.memset(ones_t, 1.0)
            bias_bf = const.tile([1, 3, D], BF16)
            for w_idx, b in enumerate((bq, bk, bv)):
                nc.gpsimd.dma_start(out=bias_bf[:, w_idx, :], in_=b[:])

        # ---- x loads prefetch (gpsimd SWDGE cast f32->bf16) ----
        x1bs, x2bs = {}, {}

        def x_load(g):
            x1b = xpool.tile([P, XG, D], BF16, tag="x1b")
            nc.gpsimd.dma_start(
                out=x1b,
                in_=in1[g * XG * P:(g + 1) * XG * P, :].rearrange(
                    "(a p) d -> p a d", p=P),
            )
            x1bs[g] = x1b
            x2b = xpool.tile([P, XG, D], BF16, tag="x2b")
            nc.gpsimd.dma_start(
                out=x2b,
                in_=in2[g * XG * P:(g + 1) * XG * P, :].rearrange(
                    "(a p) d -> p a d", p=P),
            )
            x2bs[g] = x2b

        for g in range(min(3, NG)):
            x_load(g)

        # ---- weights: stage-major emission so the in-order SP queue never
        # stalls on a not-yet-ready transpose source ----
        wnats = []
        for w_idx, W in enumerate((Wq, Wk, Wv)):
            for o_t in range(KT):
                wnat = stage.tile([P, D], F32, tag="wnat")
                nc.sync.dma_start(out=wnat, in_=W[o_t * P:(o_t + 1) * P, :])
                wnats.append(wnat)
        wbfs = []
        for i in range(12):
            wbf = wpool.tile([P, D], BF16, tag="wbf")
            nc.vector.tensor_copy(out=wbf, in_=wnats[i])
            wbfs.append(wbf)
        for w_idx in range(3):
            for o_t in range(KT):
                nc.sync.dma_start(
                    out=wT[:, w_idx, o_t, :, :],
                    in_=wbfs[w_idx * KT + o_t][:, :],
                    transpose=True,
                )

        x2Ts = {}

        def xT_stage(g):
            nc.sync.dma_start(
                out=x1T[:, g, :, :], in_=x1bs[g][:, :, :], transpose=True)
            x2T = xpool.tile([P, XG * KT, P], BF16, tag="x2T")
            nc.sync.dma_start(out=x2T, in_=x2bs[g][:, :, :], transpose=True)
            x2Ts[g] = x2T

        def x1t_lhs(n_t, i_t):
            g, a = divmod(n_t, XG)
            return x1T[:, g, a * KT + i_t, :]

        # ---- position-bias panel pipeline ----
        panels = {}

        def pb_stage(t_t):
            pbrow = pbpool.tile([P, N], F32, tag="pbrow")
            nc.sync.dma_start(out=pbrow, in_=pb[t_t * P:(t_t + 1) * P, :])
            pbexp = pbpool.tile([P, N], BF16, tag="pbexp")
            nc.scalar.activation(
                out=pbexp, in_=pbrow, func=mybir.ActivationFunctionType.Exp)
            # panel[p, s_t, f] == expB[t_t*P + f, s_t*P + p]
            panel = pbpool.tile([P, NT, P], BF16, tag="panel")
            nc.sync.dma_start(out=panel, in_=pbexp[:, :], transpose=True)
            panels[t_t] = panel

        xT_stage(0)

        for t_t in range(PRE):
            pb_stage(t_t)

        # ---- phase KV: k/v projections, exp, ekv ----
        with tc.tile_pool(name="psum_kv", bufs=2, space="PSUM") as psum_kv:
            for g in range(NG):
                if g + 3 < NG:
                    x_load(g + 3)
                if g + 1 < NG:
                    xT_stage(g + 1)
                x2T = x2Ts.pop(g)
                for a in range(XG):
                    n_t = g * XG + a
                    psk = psum_kv.tile([P, D], F32, tag="psk")
                    psv = psum_kv.tile([P, D], F32, tag="psv")
                    for i_t in range(KT):
                        nc.tensor.matmul(
                            psk,
                            x1t_lhs(n_t, i_t),
                            wT[:, 1, :, i_t, :],
                            start=(i_t == 0),
                            stop=(i_t == KT - 1 and not with_bias),
                        )
                    for i_t in range(KT):
                        nc.tensor.matmul(
                            psv,
                            x2T[:, a * KT + i_t, :],
                            wT[:, 2, :, i_t, :],
                            start=(i_t == 0),
                            stop=(i_t == KT - 1 and not with_bias),
                        )
                    if with_bias:
                        nc.tensor.matmul(psk, ones_t, bias_bf[:, 1, :],
                                         start=False, stop=True)
                        nc.tensor.matmul(psv, ones_t, bias_bf[:, 2, :],
                                         start=False, stop=True)

                    nc.scalar.activation(
                        out=ek_sb[:, n_t, :], in_=psk,
                        func=mybir.ActivationFunctionType.Exp)
                    nc.vector.tensor_mul(
                        ekv_sb[:, n_t, :], ek_sb[:, n_t, :], psv)

        # ---- phase Q: q projections + sigmoid ----
        with tc.tile_pool(name="psum_q", bufs=2, space="PSUM") as psum_q:
            for n_t in range(NT):
                psq = psum_q.tile([P, D], F32, tag="psq")
                for i_t in range(KT):
                    nc.tensor.matmul(
                        psq,
                        x1t_lhs(n_t, i_t),
                        wT[:, 0, :, i_t, :],
                        start=(i_t == 0),
                        stop=(i_t == KT - 1 and not with_bias),
                    )
                if with_bias:
                    nc.tensor.matmul(psq, ones_t, bias_bf[:, 0, :],
                                     start=False, stop=True)
                nc.scalar.activation(
                    out=qsig_sb[:, n_t, :], in_=psq,
                    func=mybir.ActivationFunctionType.Sigmoid)

        psum_nd = ctx.enter_context(
            tc.tile_pool(name="psum_nd", bufs=3, space="PSUM"))

        # ---- phase T: num/den + epilogue per t-tile ----
        for t_t in range(NT):
            if t_t + PRE < NT:
                pb_stage(t_t + PRE)
            panel = panels.pop(t_t)

            pnum = psum_nd.tile([P, D], F32, tag="pnum")
            pden = psum_nd.tile([P, D], F32, tag="pden")
            for s_t in range(NT):
                lhsT = panel[:, s_t, :]
                nc.tensor.matmul(pnum, lhsT, ekv_sb[:, s_t, :],
                                 start=(s_t == 0), stop=(s_t == NT - 1))
                nc.tensor.matmul(pden, lhsT, ek_sb[:, s_t, :],
                                 start=(s_t == 0), stop=(s_t == NT - 1))

            rec = stage.tile([P, D], F32, tag="rec")
            nc.vector.reciprocal_approx_fast(out=rec, in_=pden)
            rat = stage.tile([P, D], F32, tag="rat")
            nc.vector.tensor_mul(rat, rec, pnum)
            outt = stage.tile([P, D], F32, tag="outt")
            nc.vector.tensor_mul(outt, rat, qsig_sb[:, t_t, :])
            nc.sync.dma_start(
                out=out[t_t * P:(t_t + 1) * P, :], in_=outt)

    nc.finalize()
    return nc


def _get_nc(with_bias: bool) -> bass.Bass:
    key = with_bias
    if key not in _NC_CACHE:
        _NC_CACHE[key] = build_nc(with_bias)
    return _NC_CACHE[key]


def _make_in_maps(inputs: dict) -> list[dict]:
    in1 = np.ascontiguousarray(inputs["inputs1"], dtype=np.float32)
    in2 = np.ascontiguousarray(inputs["inputs2"], dtype=np.float32)
    shared = {
        k: np.ascontiguousarray(inputs[k], dtype=np.float32)
        for k in ("Wq", "Wk", "Wv", "bq", "bk", "bv", "position_biases")
    }
    return [
        {"inputs1": in1[c], "inputs2": in2[c], **shared}
        for c in range(BS)
    ]


def run(inputs: dict, trace: bool = False):
    """Returns (out [8,2048,512] f32, exec_time_ns or None)."""
    with_bias = any(
        np.any(np.asarray(inputs[b])) for b in ("bq", "bk", "bv"))
    nc = _get_nc(with_bias)
    in_maps = _make_in_maps(inputs)
    res = run_bass_kernel_spmd(
        nc, in_maps, core_ids=list(range(BS)), trace=trace)
    out = np.stack(
        [np.asarray(res.results[c]["out"]) for c in range(BS)], axis=0)
    return out.astype(np.float32), res.exec_time_ns


def kernel(**inputs) -> np.ndarray:
    out, _ = run(inputs, trace=False)
    return out


# revision 20
# speedup vs baseline: 2.9673x; 1.1104x over previous
"""AFT-full kernel for Trainium2, SPMD across 8 NeuronCores.

Math (per batch b):
    q = in1 @ Wq.T + bq ; k = in1 @ Wk.T + bk ; v = in2 @ Wv.T + bv
    num = exp(position_biases) @ (exp(k) * v)      # [t, d]
    den = exp(position_biases) @ exp(k)            # [t, d]
    out = sigmoid(q) * num / den

Sharding: pure data parallel — core i computes batch i (BS == 8 == n_cores).
Weights / biases / position_biases are replicated to every core.

Per-core dataflow (matmuls in bf16, accumulation in f32 PSUM):
  - GpSimd SWDGE cast-loads inputs f32->bf16; weights via SP + DVE cast.
  - All x/W DMA-xbar transposes run in ONE startup burst: every
    copy<->transpose xbar-mode transition serializes against in-flight
    DMAs (~10us), so transposes are batched and never interleaved with
    the streaming phases.
  - position_biases needs [s on partitions]; instead of transposing,
    column-panels are loaded directly with a strided DMA (1KB runs) and
    exp'd in place - zero transposes in the steady-state T phase.
  - Phase order KV -> Q -> T keeps ScalarE LUT swaps to 3 and lets Q's
    matmuls cover the KV->T dependency bubble.
  - Transpose-source tiles are write-once persist regions (a slot-reuse
    WAR race against the xbar DMA was observed on silicon).
"""

import sys

for _p in ("/opt/trn_rl_repo",):
    if _p not in sys.path:
        sys.path.insert(0, _p)

from contextlib import ExitStack

import numpy as np

import concourse.bass as bass
from concourse import bacc
import concourse.tile as tile
from concourse import mybir
from concourse.bass_utils import run_bass_kernel_spmd
from concourse.tile_rust import add_dep_helper

P = 128
N = 2048          # sequence length (n == s == t)
D = 512           # d_model
BS = 8            # batch size == number of cores
NT = N // P       # 16 row tiles
KT = D // P       # 4 contraction tiles for projections
XG = 4            # x row-tiles per transpose group
NG = NT // XG     # 4 groups
TG = 2            # t-tiles per T-phase pair
NP = NT // TG     # 8 pairs
PRE = 2           # pb pipeline depth (pairs ahead)
F32 = mybir.dt.float32
BF16 = mybir.dt.bfloat16

_NC_CACHE = {}


def build_nc(with_bias: bool, sym_pb: bool) -> bass.Bass:
    nc = bacc.Bacc()

    in1 = nc.declare_dram_parameter("inputs1", [N, D], F32, isOutput=False)
    in2 = nc.declare_dram_parameter("inputs2", [N, D], F32, isOutput=False)
    Wq = nc.declare_dram_parameter("Wq", [D, D], F32, isOutput=False)
    Wk = nc.declare_dram_parameter("Wk", [D, D], F32, isOutput=False)
    Wv = nc.declare_dram_parameter("Wv", [D, D], F32, isOutput=False)
    bq = nc.declare_dram_parameter("bq", [D], F32, isOutput=False)
    bk = nc.declare_dram_parameter("bk", [D], F32, isOutput=False)
    bv = nc.declare_dram_parameter("bv", [D], F32, isOutput=False)
    pb = nc.declare_dram_parameter("position_biases", [N, N], F32, isOutput=False)
    out = nc.declare_dram_parameter("out", [N, D], F32, isOutput=True)

    with ExitStack() as ctx:
        tc = ctx.enter_context(tile.TileContext(nc))

        persist = ctx.enter_context(tc.tile_pool(name="persist", bufs=1))
        # wT[p, w, o_t, i_t, f] == W_w[o_t*P + f, i_t*P + p]
        wT = persist.tile([P, 3, KT, KT, P], BF16)
        # x1T[p, g, a*KT + i_t, f] == in1[(g*XG + a)*P + f, i_t*P + p]
        x1T = persist.tile([P, NG, XG * KT, P], BF16)
        ek_sb = persist.tile([P, NT, D], BF16)     # exp(k), s on partitions
        ekv_sb = persist.tile([P, NT, D], BF16)    # exp(k) * v
        qsig_sb = persist.tile([P, NT, D], BF16)   # sigmoid(q)

        stage = ctx.enter_context(tc.tile_pool(name="stage", bufs=2))

        ones_t = bias_bf = None
        if with_bias:
            const = ctx.enter_context(tc.tile_pool(name="const", bufs=1))
            ones_t = const.tile([1, P], BF16)
            nc.vector.memset(ones_t, 1.0)
            bias_bf = const.tile([1, 3, D], BF16)
            for w_idx, b in enumerate((bq, bk, bv)):
                nc.gpsimd.dma_start(out=bias_bf[:, w_idx, :], in_=b[:])

        # ================= startup: loads, casts, one transpose burst,
        # then phase KV — all inside a scoped staging pool ===============
        with tc.tile_pool(name="xw", bufs=1) as xw, \
                tc.tile_pool(name="xwstage", bufs=2) as xwstage:
            # write-once staging (no slot reuse -> no WAR vs xbar reads)
            x1b = xw.tile([P, NG, XG, D], BF16)
            x2b = xw.tile([P, NG, XG, D], BF16)
            x2T = xw.tile([P, NG, XG * KT, P], BF16)
            wbf = xw.tile([P, 3, KT, D], BF16)

            # f32 staging rotates through pool slots; its reader is the DVE
            # cast (precise engine completion), so slot-reuse WAR is safe.
            # Loads split across the two HWDGE issuers (SP / ACT).
            casts = []
            for g in range(NG):
                x1f = xwstage.tile([P, XG, D], F32, tag="x1f")
                nc.sync.dma_start(
                    out=x1f,
                    in_=in1[g * XG * P:(g + 1) * XG * P, :].rearrange(
                        "(a p) d -> p a d", p=P),
                )
                x2f = xwstage.tile([P, XG, D], F32, tag="x2f")
                nc.scalar.dma_start(
                    out=x2f,
                    in_=in2[g * XG * P:(g + 1) * XG * P, :].rearrange(
                        "(a p) d -> p a d", p=P),
                )
                casts.append(nc.vector.tensor_copy(out=x1b[:, g, :, :], in_=x1f))
                casts.append(nc.vector.tensor_copy(out=x2b[:, g, :, :], in_=x2f))
            for w_idx, W in enumerate((Wq, Wk, Wv)):
                for o_t in range(KT):
                    wf = xwstage.tile([P, D], F32, tag="wf")
                    eng = nc.scalar if w_idx == 2 else nc.sync
                    eng.dma_start(
                        out=wf, in_=W[o_t * P:(o_t + 1) * P, :])
                    casts.append(nc.vector.tensor_copy(
                        out=wbf[:, w_idx, o_t, :], in_=wf))

            # ---- single xbar transpose burst: 5 whole-tensor transposes,
            # all ordered after the last cast so Tile cannot interleave
            # copy-DMAs into the burst (each xbar-mode transition drains
            # all in-flight DMAs, ~10us) ----
            last_cast = casts[-1].ins
            burst = []
            for o_t in range(KT):
                burst.append((nc.sync, wT[:, 1, o_t, :, :], wbf[:, 1, o_t, :]))
            for g in range(NG):
                burst.append((nc.sync, x1T[:, g, :, :], x1b[:, g, :, :]))
                burst.append((nc.scalar, x2T[:, g, :, :], x2b[:, g, :, :]))
            for o_t in range(KT):
                burst.append((nc.scalar, wT[:, 2, o_t, :, :], wbf[:, 2, o_t, :]))
            for o_t in range(KT):
                burst.append((nc.sync, wT[:, 0, o_t, :, :], wbf[:, 0, o_t, :]))
            for eng, out_ap, in_ap in burst:
                eng.dma_start(out=out_ap, in_=in_ap, transpose=True)

            def x1t_lhs(n_t, i_t):
                g, a = divmod(n_t, XG)
                return x1T[:, g, a * KT + i_t, :]

            # ---- phase KV: k/v projections, exp, ekv ----
            with tc.tile_pool(name="psum_kv", bufs=2, space="PSUM") as psum_kv:
                for n_t in range(NT):
                    g, a = divmod(n_t, XG)
                    psk = psum_kv.tile([P, D], F32, tag="psk")
                    psv = psum_kv.tile([P, D], F32, tag="psv")
                    for i_t in range(KT):
                        nc.tensor.matmul(
                            psk,
                            x1t_lhs(n_t, i_t),
                            wT[:, 1, :, i_t, :],
                            start=(i_t == 0),
                            stop=(i_t == KT - 1 and not with_bias),
                        )
                    for i_t in range(KT):
                        nc.tensor.matmul(
                            psv,
                            x2T[:, g, a * KT + i_t, :],
                            wT[:, 2, :, i_t, :],
                            start=(i_t == 0),
                            stop=(i_t == KT - 1 and not with_bias),
                        )
                    if with_bias:
                        nc.tensor.matmul(psk, ones_t, bias_bf[:, 1, :],
                                         start=False, stop=True)
                        nc.tensor.matmul(psv, ones_t, bias_bf[:, 2, :],
                                         start=False, stop=True)

                    nc.scalar.activation(
                        out=ek_sb[:, n_t, :], in_=psk,
                        func=mybir.ActivationFunctionType.Exp)
                    nc.vector.tensor_mul(
                        ekv_sb[:, n_t, :], ek_sb[:, n_t, :], psv)

        # ============ pb panel pipeline ==========
        pbpool = ctx.enter_context(tc.tile_pool(name="pbpool", bufs=PRE + 1))
        panels = {}

        def pb_stage(j):
            if sym_pb:
                # Column-panel load: pbcol[p, s_t, c] = pb[s_t*P + p, j*TG*P + c]
                # (1KB contiguous runs). exp gives expB[s, t] which equals the
                # needed stationary expB[t, s] because pb is symmetric.
                pbcol = pbpool.tile([P, NT, TG * P], F32, tag="pbcol")
                nc.sync.dma_start(
                    out=pbcol,
                    in_=pb[:, j * TG * P:(j + 1) * TG * P].rearrange(
                        "(st p) t -> p st t", p=P),
                )
                panel = pbpool.tile([P, NT, TG * P], BF16, tag="panel")
                nc.scalar.activation(
                    out=panel, in_=pbcol,
                    func=mybir.ActivationFunctionType.Exp)
                # lhsT for (a, s_t) = panel[:, s_t, a*P:(a+1)*P]
                panels[j] = panel
            else:
                # General path: contiguous row load, exp, one batched xbar
                # transpose per pair.
                pbrow = pbpool.tile([P, TG, N], F32, tag="pbrow")
                nc.sync.dma_start(
                    out=pbrow,
                    in_=pb[j * TG * P:(j + 1) * TG * P, :].rearrange(
                        "(a p) s -> p a s", p=P),
                )
                pbexp = pbpool.tile([P, TG, N], BF16, tag="pbexp")
                nc.scalar.activation(
                    out=pbexp, in_=pbrow,
                    func=mybir.ActivationFunctionType.Exp)
                # panel[p, a*NT + s_t, f] == expB[(j*TG+a)*P + f, s_t*P + p]
                panel = pbpool.tile([P, TG * NT, P], BF16, tag="panel")
                nc.sync.dma_start(
                    out=panel, in_=pbexp[:, :, :], transpose=True)
                panels[j] = panel

        def panel_lhs(panel, a, s_t):
            if sym_pb:
                return panel[:, s_t, a * P:(a + 1) * P]
            return panel[:, a * NT + s_t, :]

        for j in range(PRE):
            pb_stage(j)

        # ---- phase Q: q projections + sigmoid ----
        with tc.tile_pool(name="psum_q", bufs=3, space="PSUM") as psum_q:
            for n_t in range(NT):
                g, a = divmod(n_t, XG)
                psq = psum_q.tile([P, D], F32, tag="psq")
                for i_t in range(KT):
                    nc.tensor.matmul(
                        psq,
                        x1T[:, g, a * KT + i_t, :],
                        wT[:, 0, :, i_t, :],
                        start=(i_t == 0),
                        stop=(i_t == KT - 1 and not with_bias),
                    )
                if with_bias:
                    nc.tensor.matmul(psq, ones_t, bias_bf[:, 0, :],
                                     start=False, stop=True)
                nc.scalar.activation(
                    out=qsig_sb[:, n_t, :], in_=psq,
                    func=mybir.ActivationFunctionType.Sigmoid)

        psum_nd = ctx.enter_context(
            tc.tile_pool(name="psum_nd", bufs=2, space="PSUM"))

        # ---- phase T: num/den + epilogue per pair of t-tiles ----
        for j in range(NP):
            if j + PRE < NP:
                pb_stage(j + PRE)
            panel = panels.pop(j)

            pnum = psum_nd.tile([P, TG, D], F32, tag="pnum")
            pden = psum_nd.tile([P, TG, D], F32, tag="pden")
            for a in range(TG):
                for s_t in range(NT):
                    lhsT = panel_lhs(panel, a, s_t)
                    nc.tensor.matmul(pnum[:, a, :], lhsT, ekv_sb[:, s_t, :],
                                     start=(s_t == 0), stop=(s_t == NT - 1))
                    nc.tensor.matmul(pden[:, a, :], lhsT, ek_sb[:, s_t, :],
                                     start=(s_t == 0), stop=(s_t == NT - 1))

            rec = stage.tile([P, TG, D], F32, tag="rec")
            nc.vector.reciprocal_approx_fast(out=rec, in_=pden)
            rat = stage.tile([P, TG, D], F32, tag="rat")
            nc.vector.tensor_mul(rat, rec, pnum)
            outt = stage.tile([P, TG, D], F32, tag="outt")
            nc.vector.tensor_mul(outt, rat, qsig_sb[:, j * TG:(j + 1) * TG, :])
            nc.sync.dma_start(
                out=out[j * TG * P:(j + 1) * TG * P, :].rearrange(
                    "(a p) d -> p a d", p=P),
                in_=outt,
            )

    nc.finalize()
    return nc


def _get_nc(with_bias: bool, sym_pb: bool) -> bass.Bass:
    key = (with_bias, sym_pb)
    if key not in _NC_CACHE:
        _NC_CACHE[key] = build_nc(with_bias, sym_pb)
    return _NC_CACHE[key]


def _make_in_maps(inputs: dict) -> list[dict]:
    in1 = np.ascontiguousarray(inputs["inputs1"], dtype=np.float32)
    in2 = np.ascontiguousarray(inputs["inputs2"], dtype=np.float32)
    shared = {
        k: np.ascontiguousarray(inputs[k], dtype=np.float32)
        for k in ("Wq", "Wk", "Wv", "bq", "bk", "bv", "position_biases")
    }
    return [
        {"inputs1": in1[c], "inputs2": in2[c], **shared}
        for c in range(BS)
    ]


def run(inputs: dict, trace: bool = False):
    """Returns (out [8,2048,512] f32, exec_time_ns or None)."""
    with_bias = any(
        np.any(np.asarray(inputs[b])) for b in ("bq", "bk", "bv"))
    pbv = np.asarray(inputs["position_biases"])
    sym_pb = bool(np.array_equal(pbv, pbv.T))
    nc = _get_nc(with_bias, sym_pb)
    in_maps = _make_in_maps(inputs)
    res = run_bass_kernel_spmd(
        nc, in_maps, core_ids=list(range(BS)), trace=trace)
    out = np.stack(
        [np.asarray(res.results[c]["out"]) for c in range(BS)], axis=0)
    return out.astype(np.float32), res.exec_time_ns


def kernel(**inputs) -> np.ndarray:
    out, _ = run(inputs, trace=False)
    return out
